# revision 27
# baseline (speedup 1.0000x reference)
"""Self-contained Trainium2 Bass kernel for GQA attention (B=2, T=2048, D=4096,
32 q heads / 8 kv heads, HD=128, RoPE, no causal mask, start_pos=0).

Sharding: 8 cores = 2 (batch) x 4 (head groups). Each core computes 8 q heads /
2 kv heads for one batch and a partial o-projection; the host sums the 4
partials per batch.

All matmul operands are fp16 (FWL-enabled weight loads, 1024-col moving
operands); PSUM accumulation is fp32. exp uses bias=-2 so values fit fp16.

Per-core structure (transposed space, feature dim on partitions):
  Phase 1 (per 512-col t-chunk): qT = wqT.T @ xT (8 psum banks), RoPE evac to
    SBUF fp16; then kT (rope'd) + v in natural [t, hd] layout (x as stationary).
    wkv resident in SBUF; wq streamed per chunk; x chunk resident for both passes.
  Phase 2 (per head, per 1024-col chunk): scoresT = k_blk.T @ qT (N=1024),
    exp via one ACT op -> fp16 SBUF, den += ones.T @ exp, ctx += v.T @ exp,
    then ctx *= reciprocal_approx_fast(den).
  Phase 3: yT[m-block] = sum_h woT.T @ ctx_h (N=1024), evac fp16, DMA out.
"""

import sys
import math

for _p in ("/opt/trn_rl_repo", "/root/.axon_site"):
    if _p not in sys.path:
        sys.path.insert(0, _p)

import numpy as np

T = 2048
D = 4096
N_HEADS = 32
N_KV = 8
HD = 128
N_CORES = 8
GQ = N_HEADS // 4   # q heads per core = 8
GKV = N_KV // 4     # kv heads per core = 2
TCH = 512           # t-chunk for phase 1
TQ = 1024           # t-chunk for phases 2/3
SCALE = 1.0 / math.sqrt(HD)


def _build_program():
    import concourse.bass as bass
    import concourse.tile as tile
    from concourse import bacc, mybir

    f32 = mybir.dt.float32
    f16 = mybir.dt.float16
    bf16 = mybir.dt.bfloat16

    QD, KD, KT = GQ * HD, GKV * HD, D // 128
    N_REP = GQ // GKV

    nc = bacc.Bacc("TRN2", target_bir_lowering=False, debug=False,
                   num_devices=N_CORES)

    xT = nc.dram_tensor("xT", [D, T], f16, kind="ExternalInput")
    wqT = nc.dram_tensor("wqT", [D, QD], f16, kind="ExternalInput")
    wkvT = nc.dram_tensor("wkvT", [D, 2 * KD], f16, kind="ExternalInput")
    # wo pre-permuted on host: [128, m-block * (hk * 128)] so each m-block's
    # eight stationary [128,128] slices are one contiguous [128, 1024] DMA
    woT = nc.dram_tensor("woT", [128, (D // 128) * QD], f16,
                         kind="ExternalInput")
    C2 = nc.dram_tensor("C2", [128, T], f16, kind="ExternalInput")
    S2m = nc.dram_tensor("S2m", [128, T], f16, kind="ExternalInput")
    ones = nc.dram_tensor("ones", [128, 128], bf16, kind="ExternalInput")
    yT = nc.dram_tensor("yT", [D, T], f16, kind="ExternalOutput")

    NTCH = T // TCH           # 4
    NSB = T // 128            # 16 s-blocks for attention
    NTB = TCH // 128          # 4 v row-blocks per chunk
    SWAP = [(i + 16) % 32 for i in range(32)]  # swap 16-halves in each quadrant

    with tile.TileContext(nc) as tc:
        with tc.tile_pool(name="persist", bufs=1) as persist:
            # tiles allocated here; DMAs issued inside the chunk-0 loop so the
            # first q matmuls aren't stuck behind 6MB of weight/table loads
            ones_sb = persist.tile([128, 128], bf16, tag="ones")
            c2_sb = persist.tile([128, T], f16, tag="c2")
            s2m_sb = persist.tile([128, T], f16, tag="s2m")
            wkv_sb = [persist.tile([128, 2 * KD], f16, name=f"wkv{k}", tag=f"wkv{k}")
                      for k in range(KT)]
            # q / k (rope'd, [hd, t]) and v (natural [t, hd]) in SBUF fp16
            q_sb = [persist.tile([128, T], f16, name=f"q{m}", tag=f"q{m}")
                    for m in range(GQ)]
            k_sb = [persist.tile([128, T], f16, name=f"k{m}", tag=f"k{m}")
                    for m in range(GKV)]
            v_sb = [persist.tile([128, KD], bf16, name=f"v{tb}", tag=f"v{tb}")
                    for tb in range(T // 128)]

            # ---------------- Phase 1: q/k/v projections + RoPE ----------
            with tc.tile_pool(name="xt", bufs=1) as xtp, \
                 tc.tile_pool(name="wq", bufs=4) as wqp, \
                 tc.tile_pool(name="rope", bufs=4) as ropep, \
                 tc.tile_pool(name="p1ps", bufs=8, space="PSUM") as p1ps:

                def rope_evac(ps, dst_ap, tcol0, tcol1):
                    # dst = ps * C2 + shuffle(ps) * S2m  (on the chunk col slice)
                    t1 = ropep.tile([128, TCH], f32, tag="t1")
                    nc.vector.tensor_mul(t1[:], ps[:], c2_sb[:, tcol0:tcol1])
                    sh = ropep.tile([128, TCH], f32, tag="sh")
                    nc.vector.stream_shuffle(sh[:], ps[:], SWAP)
                    t2 = ropep.tile([128, TCH], f32, tag="t2")
                    nc.vector.tensor_mul(t2[:], sh[:], s2m_sb[:, tcol0:tcol1])
                    nc.vector.tensor_add(dst_ap, t1[:], t2[:])

                for tch in range(NTCH):
                    tcol0, tcol1 = tch * TCH, (tch + 1) * TCH
                    xts = []
                    # pass A: q projection (8 live PSUM accumulators)
                    qps = [p1ps.tile([128, TCH], f32, name=f"qps{_}", tag="p1")
                           for _ in range(GQ)]
                    for k in range(KT):
                        xt = xtp.tile([128, TCH], f16, tag=f"xt{k}")
                        nc.sync.dma_start(xt[:], xT[k * 128:(k + 1) * 128, tcol0:tcol1])
                        xts.append(xt)
                        wq_sl = wqp.tile([128, QD], f16, tag="wq")
                        nc.sync.dma_start(wq_sl[:], wqT[k * 128:(k + 1) * 128, :])
                        if tch == 0:
                            # trickle in the persistent tensors behind the
                            # chunk-0 x/wq loads (wkv needed in pass B, tables
                            # at first rope evac, ones in phase 2)
                            nc.sync.dma_start(wkv_sb[k][:],
                                              wkvT[k * 128:(k + 1) * 128, :])
                            if k == 20:
                                nc.sync.dma_start(c2_sb[:], C2[:])
                            if k == 26:
                                nc.sync.dma_start(s2m_sb[:], S2m[:])
                            if k == 30:
                                nc.sync.dma_start(ones_sb[:], ones[:])
                        for m in range(GQ):
                            nc.tensor.matmul(qps[m][:], wq_sl[:, m * 128:(m + 1) * 128],
                                             xt[:], start=(k == 0), stop=(k == KT - 1))
                    for m in range(GQ):
                        rope_evac(qps[m], q_sb[m][:, tcol0:tcol1], tcol0, tcol1)
                    # pass B: k and v projections (x tiles reused from SBUF)
                    kps = [p1ps.tile([128, TCH], f32, name=f"kps{_}", tag="p1")
                           for _ in range(GKV)]
                    vps = [p1ps.tile([128, KD], f32, name=f"vps{_}", tag="p1")
                           for _ in range(NTB)]
                    # kps first (needs only 2 freed PSUM slots, so pass B can
                    # start while the q rope evacs still drain); the k rope
                    # evacs then overlap the vps matmul sweep
                    for k in range(KT):
                        for m in range(GKV):
                            nc.tensor.matmul(kps[m][:], wkv_sb[k][:, m * 128:(m + 1) * 128],
                                             xts[k][:], start=(k == 0), stop=(k == KT - 1))
                    for m in range(GKV):
                        rope_evac(kps[m], k_sb[m][:, tcol0:tcol1], tcol0, tcol1)
                    for k in range(KT):
                        for tb in range(NTB):
                            nc.tensor.matmul(vps[tb][:], xts[k][:, tb * 128:(tb + 1) * 128],
                                             wkv_sb[k][:, KD:], start=(k == 0), stop=(k == KT - 1))
                    for tb in range(NTB):
                        nc.scalar.copy(v_sb[tch * NTB + tb][:], vps[tb][:])

            # ---------------- Phase 2: attention per q head --------------
            with tc.tile_pool(name="ctxsb", bufs=1) as ctxp:
              ctx_sb = [ctxp.tile([128, T], f16, name=f"ctx{h}", tag=f"ctx{h}")
                        for h in range(GQ)]
              with tc.tile_pool(name="exp", bufs=9) as expp, \
                   tc.tile_pool(name="nrm", bufs=2) as nrmp, \
                   tc.tile_pool(name="scps", bufs=2, space="PSUM") as scps, \
                   tc.tile_pool(name="ctxps", bufs=1, space="PSUM") as ctxps, \
                   tc.tile_pool(name="denps", bufs=1, space="PSUM") as denps:
                # Software pipeline over groups of 4 s-blocks: scores+exp for
                # group g+1 are interleaved with the ctx matmuls of group g, so
                # the PE never head-of-line blocks on the scalar engine's exp.
                # The den (softmax denominator) matmuls only need one output
                # row, so each group's four are packed as M=32 column tiles at
                # col positions 0/32/64/96 -- they run concurrently on the PE
                # (quadrant i of den_ps accumulates sb = i mod 4); the four
                # partial dens are summed on DVE after 3 PSUM->SBUF DMAs.
                NG = NSB // 4
                groups = [(h, tq, g) for h in range(GQ)
                          for tq in range(T // TQ) for g in range(NG)]

                def issue_one(h, tq, g, i):
                    kv = h // N_REP
                    sb = 4 * g + i
                    sc_ps = scps.tile([128, TQ], f32, name="scp", tag="sc")
                    for j in (0, 1):
                        nc.tensor.matmul(sc_ps[:, j * 512:(j + 1) * 512],
                                         k_sb[kv][:, sb * 128:(sb + 1) * 128],
                                         q_sb[h][:, tq * TQ + j * 512:tq * TQ + (j + 1) * 512],
                                         start=True, stop=True)
                    ex = expp.tile([128, TQ], bf16, name="exp", tag="ex")
                    nc.scalar.activation(ex[:], sc_ps[:],
                                         mybir.ActivationFunctionType.Exp,
                                         scale=SCALE)
                    return ex

                acc = {}   # (h, tq) -> (ctx_ps, den_ps)
                exs_pend = [issue_one(*groups[0], i) for i in range(4)]
                for gi, (h, tq, g) in enumerate(groups):
                    kv = h // N_REP
                    exs = exs_pend
                    exs_pend = []
                    if g == 0:
                        acc[(h, tq)] = (ctxps.tile([128, TQ], f32, name="ctxp", tag="ctx"),
                                        denps.tile([128, TQ], f32, name="denp", tag="den"))
                    ctx_ps, den_ps = acc[(h, tq)]
                    for i in range(4):
                        sb = 4 * g + i
                        if gi + 1 < len(groups):
                            # next group's scores+exp trickle in one sb at a
                            # time so the scalar engine never starves
                            exs_pend.append(issue_one(*groups[gi + 1], i))
                        for j in (0, 1):
                            nc.tensor.matmul(ctx_ps[:, j * 512:(j + 1) * 512],
                                             v_sb[sb][:, kv * 128:(kv + 1) * 128],
                                             exs[i][:, j * 512:(j + 1) * 512],
                                             start=(sb == 0), stop=(sb == NSB - 1))
                    # packed den: M=64 col tiles at positions 0/64, split by
                    # j-half -- row-half j accumulates columns j*512:(j+1)*512
                    # over ALL sb, so each half is a complete denominator for
                    # its columns (no cross-half fold needed). The two MMs per
                    # i are adjacent -> run concurrently on the PE.
                    for i in range(4):
                        for j in (0, 1):
                            nc.tensor.matmul(den_ps[64 * j:64 * (j + 1), j * 512:(j + 1) * 512],
                                             ones_sb[:, 0:64],
                                             exs[i][:, j * 512:(j + 1) * 512],
                                             start=(g == 0 and i == 0),
                                             stop=(g == NG - 1 and i == 3))
                    if g == NG - 1:
                        # reciprocal per j-half straight off PSUM, then
                        # normalize ctx per 64-partition half (rb lives on
                        # partitions 0:64, no broadcast needed)
                        dtmp = nrmp.tile([64, 512], f32, name="dtmp", tag="dtmp")
                        nc.vector.tensor_copy(dtmp[:], den_ps[64:128, 512:1024])
                        rb = nrmp.tile([64, TQ], f32, name="rb", tag="rb")
                        nc.vector.reciprocal_approx_fast(
                            rb[:, 0:512], den_ps[0:64, 0:512])
                        nc.vector.reciprocal_approx_fast(
                            rb[:, 512:1024], dtmp[:])
                        for pb in (0, 64):
                            nc.vector.tensor_mul(
                                ctx_sb[h][pb:pb + 64, tq * TQ:(tq + 1) * TQ],
                                ctx_ps[pb:pb + 64, :], rb[:])
                        del acc[(h, tq)]

              # ------------- Phase 3: o-projection (ctx in SBUF) -------
              with tc.tile_pool(name="wo", bufs=3) as wop, \
                   tc.tile_pool(name="out", bufs=4) as outp, \
                   tc.tile_pool(name="yps", bufs=3, space="PSUM") as yps:
                  for m in range(D // 128):
                      # all 8 head-blocks of wo for this m-block in one DMA
                      wo_m = wop.tile([128, QD], f16, tag="wo")
                      nc.sync.dma_start(wo_m[:], woT[:, m * QD:(m + 1) * QD])
                      for tq in range(T // TQ):
                          y_ps = yps.tile([128, TQ], f32, tag="y")
                          for hk in range(GQ):
                              for j in (0, 1):
                                  nc.tensor.matmul(
                                      y_ps[:, j * 512:(j + 1) * 512],
                                      wo_m[:, hk * 128:(hk + 1) * 128],
                                      ctx_sb[hk][:, tq * TQ + j * 512:tq * TQ + (j + 1) * 512],
                                      start=(hk == 0), stop=(hk == GQ - 1))
                          ot = outp.tile([128, TQ], f16, tag="ot")
                          if tq % 2 == 0:
                              nc.vector.tensor_copy(ot[:], y_ps[:])
                          else:
                              nc.scalar.copy(ot[:], y_ps[:])
                          nc.sync.dma_start(yT[m * 128:(m + 1) * 128,
                                               tq * TQ:(tq + 1) * TQ], ot[:])

    nc.compile()
    return nc


_PROGRAM = None


def _get_program():
    global _PROGRAM
    if _PROGRAM is None:
        _PROGRAM = _build_program()
    return _PROGRAM


def _rope_perm():
    """Within-head row permutation: row 32*q + i  <-  component 2*(16q+i%16)+ (i>=16)."""
    perm = np.empty(HD, dtype=np.int64)
    for q in range(4):
        for i in range(32):
            j = 16 * q + (i % 16)
            perm[32 * q + i] = 2 * j + (1 if i >= 16 else 0)
    return perm


def _host_prep(x, wq, wk, wv, wo, cos, sin):
    """Build the per-core input maps."""
    perm = _rope_perm()
    f32 = np.float32
    f16 = np.float16
    QD, KD = GQ * HD, GKV * HD

    cosT = np.ascontiguousarray(cos.T.astype(f32))   # [64, T]
    sinT = np.ascontiguousarray(sin.T.astype(f32))
    C2 = np.empty((128, T), f16)
    S2m = np.empty((128, T), f16)
    for q in range(4):
        for i in range(32):
            j = 16 * q + (i % 16)
            C2[32 * q + i] = cosT[j]
            S2m[32 * q + i] = sinT[j] if i >= 16 else -sinT[j]
    import ml_dtypes
    ones = np.ones((128, 128), ml_dtypes.bfloat16)

    in_maps = []
    for core in range(N_CORES):
        b, g = divmod(core, 4)
        qrows = np.concatenate([(8 * g + j) * HD + perm for j in range(GQ)])
        krows = np.concatenate([(2 * g + m) * HD + perm for m in range(GKV)])
        vrows = np.arange(2 * g * HD, (2 * g + 2) * HD)
        ocols = np.arange(8 * g * HD, (8 * g + 8) * HD)
        # woT_core[r, c] = wo[c, ocols[r]]; repack so m-block slices are contiguous:
        # WO2[p, m*1024 + hk*128 + c] = woT_core[hk*128 + p, m*128 + c]
        woT_core = wo[:, ocols].T.astype(f16)
        WO2 = np.ascontiguousarray(
            woT_core.reshape(GQ, 128, D // 128, 128)
            .transpose(1, 2, 0, 3).reshape(128, (D // 128) * QD))
        in_maps.append({
            "xT": np.ascontiguousarray(x[b].T.astype(f16)),
            "wqT": np.ascontiguousarray(wq[qrows].T.astype(f16)),
            "wkvT": np.ascontiguousarray(
                np.concatenate([wk[krows], wv[vrows]], axis=0).T.astype(f16)),
            "woT": WO2,
            "C2": C2, "S2m": S2m, "ones": ones,
        })
    return in_maps


def kernel(x, wq, wk, wv, wo, cache_k, cache_v, cos, sin, mask, start_pos):
    x = np.asarray(x)
    wq, wk, wv, wo = (np.asarray(a) for a in (wq, wk, wv, wo))
    cos, sin = np.asarray(cos), np.asarray(sin)
    assert int(start_pos) == 0, "kernel hardcodes start_pos == 0"
    assert x.shape == (2, T, D)

    from concourse.bass_utils import run_bass_kernel_spmd

    nc = _get_program()
    in_maps = _host_prep(x, wq, wk, wv, wo, cos, sin)
    res = run_bass_kernel_spmd(nc, in_maps, list(range(N_CORES)))

    y = np.empty((2, T, D), np.float32)
    for b in range(2):
        acc = res.results[4 * b]["yT"].astype(np.float32)
        for g in range(1, 4):
            acc += res.results[4 * b + g]["yT"].astype(np.float32)
        y[b] = acc.T
    return y


# revision 28
# speedup vs baseline: 1.0135x; 1.0135x over previous
"""Self-contained Trainium2 Bass kernel for GQA attention (B=2, T=2048, D=4096,
32 q heads / 8 kv heads, HD=128, RoPE, no causal mask, start_pos=0).

Sharding: 8 cores = 2 (batch) x 4 (head groups). Each core computes 8 q heads /
2 kv heads for one batch and a partial o-projection; the host sums the 4
partials per batch.

All matmul operands are fp16 (FWL-enabled weight loads, 1024-col moving
operands); PSUM accumulation is fp32. exp uses bias=-2 so values fit fp16.

Per-core structure (transposed space, feature dim on partitions):
  Phase 1 (per 512-col t-chunk): qT = wqT.T @ xT (8 psum banks), RoPE evac to
    SBUF fp16; then kT (rope'd) + v in natural [t, hd] layout (x as stationary).
    wkv resident in SBUF; wq streamed per chunk; x chunk resident for both passes.
  Phase 2 (per head, per 1024-col chunk): scoresT = k_blk.T @ qT (N=1024),
    exp via one ACT op -> fp16 SBUF, den += ones.T @ exp, ctx += v.T @ exp,
    then ctx *= reciprocal_approx_fast(den).
  Phase 3: yT[m-block] = sum_h woT.T @ ctx_h (N=1024), evac fp16, DMA out.
"""

import sys
import math

for _p in ("/opt/trn_rl_repo", "/root/.axon_site"):
    if _p not in sys.path:
        sys.path.insert(0, _p)

import numpy as np

T = 2048
D = 4096
N_HEADS = 32
N_KV = 8
HD = 128
N_CORES = 8
GQ = N_HEADS // 4   # q heads per core = 8
GKV = N_KV // 4     # kv heads per core = 2
TCH = 512           # t-chunk for phase 1
TQ = 1024           # t-chunk for phases 2/3
SCALE = 1.0 / math.sqrt(HD)


def _build_program():
    import concourse.bass as bass
    import concourse.tile as tile
    from concourse import bacc, mybir

    f32 = mybir.dt.float32
    f16 = mybir.dt.float16
    bf16 = mybir.dt.bfloat16

    QD, KD, KT = GQ * HD, GKV * HD, D // 128
    N_REP = GQ // GKV

    nc = bacc.Bacc("TRN2", target_bir_lowering=False, debug=False,
                   num_devices=N_CORES)

    xT = nc.dram_tensor("xT", [D, T], f16, kind="ExternalInput")
    wqT = nc.dram_tensor("wqT", [D, QD], f16, kind="ExternalInput")
    wkvT = nc.dram_tensor("wkvT", [D, 2 * KD], f16, kind="ExternalInput")
    # wo pre-permuted on host: [128, m-block * (hk * 128)] so each m-block's
    # eight stationary [128,128] slices are one contiguous [128, 1024] DMA
    woT = nc.dram_tensor("woT", [128, (D // 128) * QD], f16,
                         kind="ExternalInput")
    C2 = nc.dram_tensor("C2", [128, T], f16, kind="ExternalInput")
    S2m = nc.dram_tensor("S2m", [128, T], f16, kind="ExternalInput")
    ones = nc.dram_tensor("ones", [128, 128], bf16, kind="ExternalInput")
    yT = nc.dram_tensor("yT", [D, T], f16, kind="ExternalOutput")

    NTCH = T // TCH           # 4
    NSB = T // 128            # 16 s-blocks for attention
    NTB = TCH // 128          # 4 v row-blocks per chunk
    SWAP = [(i + 16) % 32 for i in range(32)]  # swap 16-halves in each quadrant

    with tile.TileContext(nc) as tc:
        with tc.tile_pool(name="persist", bufs=1) as persist:
            # tiles allocated here; DMAs issued inside the chunk-0 loop so the
            # first q matmuls aren't stuck behind 6MB of weight/table loads
            ones_sb = persist.tile([128, 128], bf16, tag="ones")
            c2_sb = persist.tile([128, T], f16, tag="c2")
            s2m_sb = persist.tile([128, T], f16, tag="s2m")
            wkv_sb = [persist.tile([128, 2 * KD], f16, name=f"wkv{k}", tag=f"wkv{k}")
                      for k in range(KT)]
            # q / k (rope'd, [hd, t]) and v (natural [t, hd]) in SBUF fp16
            q_sb = [persist.tile([128, T], f16, name=f"q{m}", tag=f"q{m}")
                    for m in range(GQ)]
            k_sb = [persist.tile([128, T], f16, name=f"k{m}", tag=f"k{m}")
                    for m in range(GKV)]
            v_sb = [persist.tile([128, KD], bf16, name=f"v{tb}", tag=f"v{tb}")
                    for tb in range(T // 128)]

            # ---------------- Phase 1: q/k/v projections + RoPE ----------
            with tc.tile_pool(name="xt", bufs=1) as xtp, \
                 tc.tile_pool(name="wq", bufs=4) as wqp, \
                 tc.tile_pool(name="rope", bufs=4) as ropep, \
                 tc.tile_pool(name="p1ps", bufs=8, space="PSUM") as p1ps:

                def rope_evac(ps, dst_ap, tcol0, tcol1):
                    # dst = ps * C2 + shuffle(ps) * S2m  (on the chunk col slice)
                    t1 = ropep.tile([128, TCH], f32, tag="t1")
                    nc.vector.tensor_mul(t1[:], ps[:], c2_sb[:, tcol0:tcol1])
                    sh = ropep.tile([128, TCH], f32, tag="sh")
                    nc.vector.stream_shuffle(sh[:], ps[:], SWAP)
                    t2 = ropep.tile([128, TCH], f32, tag="t2")
                    nc.vector.tensor_mul(t2[:], sh[:], s2m_sb[:, tcol0:tcol1])
                    nc.vector.tensor_add(dst_ap, t1[:], t2[:])

                for tch in range(NTCH):
                    tcol0, tcol1 = tch * TCH, (tch + 1) * TCH
                    xts = []
                    # pass A: q projection (8 live PSUM accumulators)
                    qps = [p1ps.tile([128, TCH], f32, name=f"qps{_}", tag="p1")
                           for _ in range(GQ)]
                    for k in range(KT):
                        xt = xtp.tile([128, TCH], f16, tag=f"xt{k}")
                        nc.sync.dma_start(xt[:], xT[k * 128:(k + 1) * 128, tcol0:tcol1])
                        xts.append(xt)
                        wq_sl = wqp.tile([128, QD], f16, tag="wq")
                        nc.sync.dma_start(wq_sl[:], wqT[k * 128:(k + 1) * 128, :])
                        if tch == 0:
                            # trickle in the persistent tensors behind the
                            # chunk-0 x/wq loads (wkv needed in pass B, tables
                            # at first rope evac, ones in phase 2)
                            nc.sync.dma_start(wkv_sb[k][:],
                                              wkvT[k * 128:(k + 1) * 128, :])
                            if k == 20:
                                nc.sync.dma_start(c2_sb[:], C2[:])
                            if k == 26:
                                nc.sync.dma_start(s2m_sb[:], S2m[:])
                            if k == 30:
                                nc.sync.dma_start(ones_sb[:], ones[:])
                        for m in range(GQ):
                            nc.tensor.matmul(qps[m][:], wq_sl[:, m * 128:(m + 1) * 128],
                                             xt[:], start=(k == 0), stop=(k == KT - 1))
                    for m in range(GQ):
                        rope_evac(qps[m], q_sb[m][:, tcol0:tcol1], tcol0, tcol1)
                    # pass B: k and v projections (x tiles reused from SBUF)
                    kps = [p1ps.tile([128, TCH], f32, name=f"kps{_}", tag="p1")
                           for _ in range(GKV)]
                    vps = [p1ps.tile([128, KD], f32, name=f"vps{_}", tag="p1")
                           for _ in range(NTB)]
                    for k in range(KT):
                        for m in range(GKV):
                            nc.tensor.matmul(kps[m][:], wkv_sb[k][:, m * 128:(m + 1) * 128],
                                             xts[k][:], start=(k == 0), stop=(k == KT - 1))
                        for tb in range(NTB):
                            nc.tensor.matmul(vps[tb][:], xts[k][:, tb * 128:(tb + 1) * 128],
                                             wkv_sb[k][:, KD:], start=(k == 0), stop=(k == KT - 1))
                    for m in range(GKV):
                        rope_evac(kps[m], k_sb[m][:, tcol0:tcol1], tcol0, tcol1)
                    for tb in range(NTB):
                        nc.scalar.copy(v_sb[tch * NTB + tb][:], vps[tb][:])

            # ---------------- Phase 2: attention per q head --------------
            with tc.tile_pool(name="ctxsb", bufs=1) as ctxp:
              ctx_sb = [ctxp.tile([128, T], f16, name=f"ctx{h}", tag=f"ctx{h}")
                        for h in range(GQ)]
              with tc.tile_pool(name="exp", bufs=9) as expp, \
                   tc.tile_pool(name="nrm", bufs=2) as nrmp, \
                   tc.tile_pool(name="scps", bufs=2, space="PSUM") as scps, \
                   tc.tile_pool(name="ctxps", bufs=1, space="PSUM") as ctxps, \
                   tc.tile_pool(name="denps", bufs=1, space="PSUM") as denps:
                # Software pipeline over groups of 4 s-blocks: scores+exp for
                # group g+1 are interleaved with the ctx matmuls of group g, so
                # the PE never head-of-line blocks on the scalar engine's exp.
                # The den (softmax denominator) matmuls only need one output
                # row, so each group's four are packed as M=32 column tiles at
                # col positions 0/32/64/96 -- they run concurrently on the PE
                # (quadrant i of den_ps accumulates sb = i mod 4); the four
                # partial dens are summed on DVE after 3 PSUM->SBUF DMAs.
                NG = NSB // 4
                groups = [(h, tq, g) for h in range(GQ)
                          for tq in range(T // TQ) for g in range(NG)]

                def issue_one(h, tq, g, i):
                    kv = h // N_REP
                    sb = 4 * g + i
                    sc_ps = scps.tile([128, TQ], f32, name="scp", tag="sc")
                    for j in (0, 1):
                        nc.tensor.matmul(sc_ps[:, j * 512:(j + 1) * 512],
                                         k_sb[kv][:, sb * 128:(sb + 1) * 128],
                                         q_sb[h][:, tq * TQ + j * 512:tq * TQ + (j + 1) * 512],
                                         start=True, stop=True)
                    ex = expp.tile([128, TQ], bf16, name="exp", tag="ex")
                    nc.scalar.activation(ex[:], sc_ps[:],
                                         mybir.ActivationFunctionType.Exp,
                                         scale=SCALE)
                    return ex

                acc = {}   # (h, tq) -> (ctx_ps, den_ps)
                exs_pend = [issue_one(*groups[0], i) for i in range(4)]
                for gi, (h, tq, g) in enumerate(groups):
                    kv = h // N_REP
                    exs = exs_pend
                    exs_pend = []
                    if g == 0:
                        acc[(h, tq)] = (ctxps.tile([128, TQ], f32, name="ctxp", tag="ctx"),
                                        denps.tile([128, TQ], f32, name="denp", tag="den"))
                    ctx_ps, den_ps = acc[(h, tq)]
                    for i in range(4):
                        sb = 4 * g + i
                        if gi + 1 < len(groups):
                            # next group's scores+exp trickle in one sb at a
                            # time so the scalar engine never starves
                            exs_pend.append(issue_one(*groups[gi + 1], i))
                        for j in (0, 1):
                            nc.tensor.matmul(ctx_ps[:, j * 512:(j + 1) * 512],
                                             v_sb[sb][:, kv * 128:(kv + 1) * 128],
                                             exs[i][:, j * 512:(j + 1) * 512],
                                             start=(sb == 0), stop=(sb == NSB - 1))
                    # packed den: M=64 col tiles at positions 0/64, split by
                    # j-half -- row-half j accumulates columns j*512:(j+1)*512
                    # over ALL sb, so each half is a complete denominator for
                    # its columns (no cross-half fold needed). The two MMs per
                    # i are adjacent -> run concurrently on the PE.
                    for i in range(4):
                        for j in (0, 1):
                            nc.tensor.matmul(den_ps[64 * j:64 * (j + 1), j * 512:(j + 1) * 512],
                                             ones_sb[:, 0:64],
                                             exs[i][:, j * 512:(j + 1) * 512],
                                             start=(g == 0 and i == 0),
                                             stop=(g == NG - 1 and i == 3))
                    if g == NG - 1:
                        # reciprocal per j-half straight off PSUM, then
                        # normalize ctx per 64-partition half (rb lives on
                        # partitions 0:64, no broadcast needed)
                        dtmp = nrmp.tile([64, 512], f32, name="dtmp", tag="dtmp")
                        nc.vector.tensor_copy(dtmp[:], den_ps[64:128, 512:1024])
                        rb = nrmp.tile([64, TQ], f32, name="rb", tag="rb")
                        nc.vector.reciprocal_approx_fast(
                            rb[:, 0:512], den_ps[0:64, 0:512])
                        nc.vector.reciprocal_approx_fast(
                            rb[:, 512:1024], dtmp[:])
                        for pb in (0, 64):
                            nc.vector.tensor_mul(
                                ctx_sb[h][pb:pb + 64, tq * TQ:(tq + 1) * TQ],
                                ctx_ps[pb:pb + 64, :], rb[:])
                        del acc[(h, tq)]

              # ------------- Phase 3: o-projection (ctx in SBUF) -------
              with tc.tile_pool(name="wo", bufs=3) as wop, \
                   tc.tile_pool(name="out", bufs=4) as outp, \
                   tc.tile_pool(name="yps", bufs=3, space="PSUM") as yps:
                  for m in range(D // 128):
                      # all 8 head-blocks of wo for this m-block in one DMA
                      wo_m = wop.tile([128, QD], f16, tag="wo")
                      nc.sync.dma_start(wo_m[:], woT[:, m * QD:(m + 1) * QD])
                      for tq in range(T // TQ):
                          y_ps = yps.tile([128, TQ], f32, tag="y")
                          for hk in range(GQ):
                              for j in (0, 1):
                                  nc.tensor.matmul(
                                      y_ps[:, j * 512:(j + 1) * 512],
                                      wo_m[:, hk * 128:(hk + 1) * 128],
                                      ctx_sb[hk][:, tq * TQ + j * 512:tq * TQ + (j + 1) * 512],
                                      start=(hk == 0), stop=(hk == GQ - 1))
                          ot = outp.tile([128, TQ], f16, tag="ot")
                          if tq % 2 == 0:
                              nc.vector.tensor_copy(ot[:], y_ps[:])
                          else:
                              nc.scalar.copy(ot[:], y_ps[:])
                          nc.sync.dma_start(yT[m * 128:(m + 1) * 128,
                                               tq * TQ:(tq + 1) * TQ], ot[:])

    nc.compile()
    return nc


_PROGRAM = None


def _get_program():
    global _PROGRAM
    if _PROGRAM is None:
        _PROGRAM = _build_program()
    return _PROGRAM


def _rope_perm():
    """Within-head row permutation: row 32*q + i  <-  component 2*(16q+i%16)+ (i>=16)."""
    perm = np.empty(HD, dtype=np.int64)
    for q in range(4):
        for i in range(32):
            j = 16 * q + (i % 16)
            perm[32 * q + i] = 2 * j + (1 if i >= 16 else 0)
    return perm


def _host_prep(x, wq, wk, wv, wo, cos, sin):
    """Build the per-core input maps."""
    perm = _rope_perm()
    f32 = np.float32
    f16 = np.float16
    QD, KD = GQ * HD, GKV * HD

    cosT = np.ascontiguousarray(cos.T.astype(f32))   # [64, T]
    sinT = np.ascontiguousarray(sin.T.astype(f32))
    C2 = np.empty((128, T), f16)
    S2m = np.empty((128, T), f16)
    for q in range(4):
        for i in range(32):
            j = 16 * q + (i % 16)
            C2[32 * q + i] = cosT[j]
            S2m[32 * q + i] = sinT[j] if i >= 16 else -sinT[j]
    import ml_dtypes
    ones = np.ones((128, 128), ml_dtypes.bfloat16)

    in_maps = []
    for core in range(N_CORES):
        b, g = divmod(core, 4)
        qrows = np.concatenate([(8 * g + j) * HD + perm for j in range(GQ)])
        krows = np.concatenate([(2 * g + m) * HD + perm for m in range(GKV)])
        vrows = np.arange(2 * g * HD, (2 * g + 2) * HD)
        ocols = np.arange(8 * g * HD, (8 * g + 8) * HD)
        # woT_core[r, c] = wo[c, ocols[r]]; repack so m-block slices are contiguous:
        # WO2[p, m*1024 + hk*128 + c] = woT_core[hk*128 + p, m*128 + c]
        woT_core = wo[:, ocols].T.astype(f16)
        WO2 = np.ascontiguousarray(
            woT_core.reshape(GQ, 128, D // 128, 128)
            .transpose(1, 2, 0, 3).reshape(128, (D // 128) * QD))
        in_maps.append({
            "xT": np.ascontiguousarray(x[b].T.astype(f16)),
            "wqT": np.ascontiguousarray(wq[qrows].T.astype(f16)),
            "wkvT": np.ascontiguousarray(
                np.concatenate([wk[krows], wv[vrows]], axis=0).T.astype(f16)),
            "woT": WO2,
            "C2": C2, "S2m": S2m, "ones": ones,
        })
    return in_maps


def kernel(x, wq, wk, wv, wo, cache_k, cache_v, cos, sin, mask, start_pos):
    x = np.asarray(x)
    wq, wk, wv, wo = (np.asarray(a) for a in (wq, wk, wv, wo))
    cos, sin = np.asarray(cos), np.asarray(sin)
    assert int(start_pos) == 0, "kernel hardcodes start_pos == 0"
    assert x.shape == (2, T, D)

    from concourse.bass_utils import run_bass_kernel_spmd

    nc = _get_program()
    in_maps = _host_prep(x, wq, wk, wv, wo, cos, sin)
    res = run_bass_kernel_spmd(nc, in_maps, list(range(N_CORES)))

    y = np.empty((2, T, D), np.float32)
    for b in range(2):
        acc = res.results[4 * b]["yT"].astype(np.float32)
        for g in range(1, 4):
            acc += res.results[4 * b + g]["yT"].astype(np.float32)
        y[b] = acc.T
    return y


# revision 30
# speedup vs baseline: 1.0256x; 1.0119x over previous
"""Self-contained Trainium2 Bass kernel for GQA attention (B=2, T=2048, D=4096,
32 q heads / 8 kv heads, HD=128, RoPE, no causal mask, start_pos=0).

Sharding: 8 cores = 2 (batch) x 4 (head groups). Each core computes 8 q heads /
2 kv heads for one batch and a partial o-projection; the host sums the 4
partials per batch.

All matmul operands are fp16 (FWL-enabled weight loads, 1024-col moving
operands); PSUM accumulation is fp32. exp uses bias=-2 so values fit fp16.

Per-core structure (transposed space, feature dim on partitions):
  Phase 1 (per 512-col t-chunk): qT = wqT.T @ xT (8 psum banks), RoPE evac to
    SBUF fp16; then kT (rope'd) + v in natural [t, hd] layout (x as stationary).
    wkv resident in SBUF; wq streamed per chunk; x chunk resident for both passes.
  Phase 2 (per head, per 1024-col chunk): scoresT = k_blk.T @ qT (N=1024),
    exp via one ACT op -> fp16 SBUF, den += ones.T @ exp, ctx += v.T @ exp,
    then ctx *= reciprocal_approx_fast(den).
  Phase 3: yT[m-block] = sum_h woT.T @ ctx_h (N=1024), evac fp16, DMA out.
"""

import sys
import math

for _p in ("/opt/trn_rl_repo", "/root/.axon_site"):
    if _p not in sys.path:
        sys.path.insert(0, _p)

import numpy as np

T = 2048
D = 4096
N_HEADS = 32
N_KV = 8
HD = 128
N_CORES = 8
GQ = N_HEADS // 4   # q heads per core = 8
GKV = N_KV // 4     # kv heads per core = 2
TCH = 512           # t-chunk for phase 1
TQ = 1024           # t-chunk for phases 2/3
SCALE = 1.0 / math.sqrt(HD)


def _build_program():
    import concourse.bass as bass
    import concourse.tile as tile
    from concourse import bacc, mybir

    f32 = mybir.dt.float32
    f16 = mybir.dt.float16
    bf16 = mybir.dt.bfloat16

    QD, KD, KT = GQ * HD, GKV * HD, D // 128
    N_REP = GQ // GKV

    nc = bacc.Bacc("TRN2", target_bir_lowering=False, debug=False,
                   num_devices=N_CORES)

    xT = nc.dram_tensor("xT", [D, T], f16, kind="ExternalInput")
    wqT = nc.dram_tensor("wqT", [D, QD], f16, kind="ExternalInput")
    wkvT = nc.dram_tensor("wkvT", [D, 2 * KD], f16, kind="ExternalInput")
    # wo pre-permuted on host: [128, m-block * (hk * 128)] so each m-block's
    # eight stationary [128,128] slices are one contiguous [128, 1024] DMA
    woT = nc.dram_tensor("woT", [128, (D // 128) * QD], f16,
                         kind="ExternalInput")
    C2 = nc.dram_tensor("C2", [128, T], f16, kind="ExternalInput")
    S2m = nc.dram_tensor("S2m", [128, T], f16, kind="ExternalInput")
    ones = nc.dram_tensor("ones", [128, 128], bf16, kind="ExternalInput")
    yT = nc.dram_tensor("yT", [D, T], f16, kind="ExternalOutput")

    NTCH = T // TCH           # 4
    NSB = T // 128            # 16 s-blocks for attention
    NTB = TCH // 128          # 4 v row-blocks per chunk
    SWAP = [(i + 16) % 32 for i in range(32)]  # swap 16-halves in each quadrant

    with tile.TileContext(nc) as tc:
        with tc.tile_pool(name="persist", bufs=1) as persist:
            # tiles allocated here; DMAs issued inside the chunk-0 loop so the
            # first q matmuls aren't stuck behind 6MB of weight/table loads
            ones_sb = persist.tile([128, 128], bf16, tag="ones")
            c2_sb = persist.tile([128, T], f16, tag="c2")
            s2m_sb = persist.tile([128, T], f16, tag="s2m")
            wkv_sb = [persist.tile([128, 2 * KD], f16, name=f"wkv{k}", tag=f"wkv{k}")
                      for k in range(KT)]
            # wq fully resident after chunk 0 (8.4MB fp16): chunks 1-3 run
            # pass A with no weight DMA at all
            wq_sb = [persist.tile([128, QD], f16, name=f"wq{k}", tag=f"wq{k}")
                     for k in range(KT)]
            # q / k (rope'd, [hd, t]) and v (natural [t, hd]) in SBUF fp16
            q_sb = [persist.tile([128, T], f16, name=f"q{m}", tag=f"q{m}")
                    for m in range(GQ)]
            k_sb = [persist.tile([128, T], f16, name=f"k{m}", tag=f"k{m}")
                    for m in range(GKV)]
            v_sb = [persist.tile([128, KD], bf16, name=f"v{tb}", tag=f"v{tb}")
                    for tb in range(T // 128)]

            # ---------------- Phase 1: q/k/v projections + RoPE ----------
            with tc.tile_pool(name="xt", bufs=1) as xtp, \
                 tc.tile_pool(name="rope", bufs=2) as ropep, \
                 tc.tile_pool(name="p1ps", bufs=8, space="PSUM") as p1ps:

                def rope_evac(ps, dst_ap, tcol0, tcol1):
                    # dst = ps * C2 + shuffle(ps) * S2m  (on the chunk col slice)
                    t1 = ropep.tile([128, TCH], f32, tag="t1")
                    nc.vector.tensor_mul(t1[:], ps[:], c2_sb[:, tcol0:tcol1])
                    sh = ropep.tile([128, TCH], f32, tag="sh")
                    nc.vector.stream_shuffle(sh[:], ps[:], SWAP)
                    t2 = ropep.tile([128, TCH], f32, tag="t2")
                    nc.vector.tensor_mul(t2[:], sh[:], s2m_sb[:, tcol0:tcol1])
                    nc.vector.tensor_add(dst_ap, t1[:], t2[:])

                for tch in range(NTCH):
                    tcol0, tcol1 = tch * TCH, (tch + 1) * TCH
                    xts = []
                    # pass A: q projection (8 live PSUM accumulators)
                    qps = [p1ps.tile([128, TCH], f32, name=f"qps{_}", tag="p1")
                           for _ in range(GQ)]
                    for k in range(KT):
                        xt = xtp.tile([128, TCH], f16, tag=f"xt{k}")
                        nc.sync.dma_start(xt[:], xT[k * 128:(k + 1) * 128, tcol0:tcol1])
                        xts.append(xt)
                        wq_sl = wq_sb[k]
                        if tch == 0:
                            nc.sync.dma_start(wq_sl[:], wqT[k * 128:(k + 1) * 128, :])
                            # trickle in the persistent tensors behind the
                            # chunk-0 x/wq loads (wkv needed in pass B, tables
                            # at first rope evac, ones in phase 2)
                            nc.sync.dma_start(wkv_sb[k][:],
                                              wkvT[k * 128:(k + 1) * 128, :])
                            if k == 20:
                                nc.sync.dma_start(c2_sb[:], C2[:])
                            if k == 26:
                                nc.sync.dma_start(s2m_sb[:], S2m[:])
                            if k == 30:
                                nc.sync.dma_start(ones_sb[:], ones[:])
                        for m in range(GQ):
                            nc.tensor.matmul(qps[m][:], wq_sl[:, m * 128:(m + 1) * 128],
                                             xt[:], start=(k == 0), stop=(k == KT - 1))
                    for m in range(GQ):
                        rope_evac(qps[m], q_sb[m][:, tcol0:tcol1], tcol0, tcol1)
                    # pass B: k and v projections (x tiles reused from SBUF)
                    kps = [p1ps.tile([128, TCH], f32, name=f"kps{_}", tag="p1")
                           for _ in range(GKV)]
                    vps = [p1ps.tile([128, KD], f32, name=f"vps{_}", tag="p1")
                           for _ in range(NTB)]
                    for k in range(KT):
                        for m in range(GKV):
                            nc.tensor.matmul(kps[m][:], wkv_sb[k][:, m * 128:(m + 1) * 128],
                                             xts[k][:], start=(k == 0), stop=(k == KT - 1))
                        for tb in range(NTB):
                            nc.tensor.matmul(vps[tb][:], xts[k][:, tb * 128:(tb + 1) * 128],
                                             wkv_sb[k][:, KD:], start=(k == 0), stop=(k == KT - 1))
                    for m in range(GKV):
                        rope_evac(kps[m], k_sb[m][:, tcol0:tcol1], tcol0, tcol1)
                    for tb in range(NTB):
                        nc.scalar.copy(v_sb[tch * NTB + tb][:], vps[tb][:])

            # ---------------- Phase 2: attention per q head --------------
            with tc.tile_pool(name="ctxsb", bufs=1) as ctxp:
              ctx_sb = [ctxp.tile([128, T], f16, name=f"ctx{h}", tag=f"ctx{h}")
                        for h in range(GQ)]
              with tc.tile_pool(name="exp", bufs=8) as expp, \
                   tc.tile_pool(name="nrm", bufs=1) as nrmp, \
                   tc.tile_pool(name="scps", bufs=2, space="PSUM") as scps, \
                   tc.tile_pool(name="ctxps", bufs=1, space="PSUM") as ctxps, \
                   tc.tile_pool(name="denps", bufs=1, space="PSUM") as denps:
                # Software pipeline over groups of 4 s-blocks: scores+exp for
                # group g+1 are interleaved with the ctx matmuls of group g, so
                # the PE never head-of-line blocks on the scalar engine's exp.
                # The den (softmax denominator) matmuls only need one output
                # row, so each group's four are packed as M=32 column tiles at
                # col positions 0/32/64/96 -- they run concurrently on the PE
                # (quadrant i of den_ps accumulates sb = i mod 4); the four
                # partial dens are summed on DVE after 3 PSUM->SBUF DMAs.
                NG = NSB // 4
                groups = [(h, tq, g) for h in range(GQ)
                          for tq in range(T // TQ) for g in range(NG)]

                def issue_one(h, tq, g, i):
                    kv = h // N_REP
                    sb = 4 * g + i
                    sc_ps = scps.tile([128, TQ], f32, name="scp", tag="sc")
                    for j in (0, 1):
                        nc.tensor.matmul(sc_ps[:, j * 512:(j + 1) * 512],
                                         k_sb[kv][:, sb * 128:(sb + 1) * 128],
                                         q_sb[h][:, tq * TQ + j * 512:tq * TQ + (j + 1) * 512],
                                         start=True, stop=True)
                    ex = expp.tile([128, TQ], bf16, name="exp", tag="ex")
                    nc.scalar.activation(ex[:], sc_ps[:],
                                         mybir.ActivationFunctionType.Exp,
                                         scale=SCALE)
                    return ex

                acc = {}   # (h, tq) -> (ctx_ps, den_ps)
                exs_pend = [issue_one(*groups[0], i) for i in range(4)]
                for gi, (h, tq, g) in enumerate(groups):
                    kv = h // N_REP
                    exs = exs_pend
                    exs_pend = []
                    if g == 0:
                        acc[(h, tq)] = (ctxps.tile([128, TQ], f32, name="ctxp", tag="ctx"),
                                        denps.tile([128, TQ], f32, name="denp", tag="den"))
                    ctx_ps, den_ps = acc[(h, tq)]
                    for i in range(4):
                        sb = 4 * g + i
                        if gi + 1 < len(groups):
                            # next group's scores+exp trickle in one sb at a
                            # time so the scalar engine never starves
                            exs_pend.append(issue_one(*groups[gi + 1], i))
                        for j in (0, 1):
                            nc.tensor.matmul(ctx_ps[:, j * 512:(j + 1) * 512],
                                             v_sb[sb][:, kv * 128:(kv + 1) * 128],
                                             exs[i][:, j * 512:(j + 1) * 512],
                                             start=(sb == 0), stop=(sb == NSB - 1))
                    # packed den: M=64 col tiles at positions 0/64, split by
                    # j-half -- row-half j accumulates columns j*512:(j+1)*512
                    # over ALL sb, so each half is a complete denominator for
                    # its columns (no cross-half fold needed). The two MMs per
                    # i are adjacent -> run concurrently on the PE.
                    for i in range(4):
                        for j in (0, 1):
                            nc.tensor.matmul(den_ps[64 * j:64 * (j + 1), j * 512:(j + 1) * 512],
                                             ones_sb[:, 0:64],
                                             exs[i][:, j * 512:(j + 1) * 512],
                                             start=(g == 0 and i == 0),
                                             stop=(g == NG - 1 and i == 3))
                    if g == NG - 1:
                        # reciprocal per j-half straight off PSUM, then
                        # normalize ctx per 64-partition half (rb lives on
                        # partitions 0:64, no broadcast needed)
                        dtmp = nrmp.tile([64, 512], f32, name="dtmp", tag="dtmp")
                        nc.vector.tensor_copy(dtmp[:], den_ps[64:128, 512:1024])
                        rb = nrmp.tile([64, TQ], f32, name="rb", tag="rb")
                        nc.vector.reciprocal_approx_fast(
                            rb[:, 0:512], den_ps[0:64, 0:512])
                        nc.vector.reciprocal_approx_fast(
                            rb[:, 512:1024], dtmp[:])
                        for pb in (0, 64):
                            nc.vector.tensor_mul(
                                ctx_sb[h][pb:pb + 64, tq * TQ:(tq + 1) * TQ],
                                ctx_ps[pb:pb + 64, :], rb[:])
                        del acc[(h, tq)]

              # ------------- Phase 3: o-projection (ctx in SBUF) -------
              with tc.tile_pool(name="wo", bufs=3) as wop, \
                   tc.tile_pool(name="out", bufs=4) as outp, \
                   tc.tile_pool(name="yps", bufs=3, space="PSUM") as yps:
                  for m in range(D // 128):
                      # all 8 head-blocks of wo for this m-block in one DMA
                      wo_m = wop.tile([128, QD], f16, tag="wo")
                      nc.sync.dma_start(wo_m[:], woT[:, m * QD:(m + 1) * QD])
                      for tq in range(T // TQ):
                          y_ps = yps.tile([128, TQ], f32, tag="y")
                          for hk in range(GQ):
                              for j in (0, 1):
                                  nc.tensor.matmul(
                                      y_ps[:, j * 512:(j + 1) * 512],
                                      wo_m[:, hk * 128:(hk + 1) * 128],
                                      ctx_sb[hk][:, tq * TQ + j * 512:tq * TQ + (j + 1) * 512],
                                      start=(hk == 0), stop=(hk == GQ - 1))
                          ot = outp.tile([128, TQ], f16, tag="ot")
                          if tq % 2 == 0:
                              nc.vector.tensor_copy(ot[:], y_ps[:])
                          else:
                              nc.scalar.copy(ot[:], y_ps[:])
                          nc.sync.dma_start(yT[m * 128:(m + 1) * 128,
                                               tq * TQ:(tq + 1) * TQ], ot[:])

    nc.compile()
    return nc


_PROGRAM = None


def _get_program():
    global _PROGRAM
    if _PROGRAM is None:
        _PROGRAM = _build_program()
    return _PROGRAM


def _rope_perm():
    """Within-head row permutation: row 32*q + i  <-  component 2*(16q+i%16)+ (i>=16)."""
    perm = np.empty(HD, dtype=np.int64)
    for q in range(4):
        for i in range(32):
            j = 16 * q + (i % 16)
            perm[32 * q + i] = 2 * j + (1 if i >= 16 else 0)
    return perm


def _host_prep(x, wq, wk, wv, wo, cos, sin):
    """Build the per-core input maps."""
    perm = _rope_perm()
    f32 = np.float32
    f16 = np.float16
    QD, KD = GQ * HD, GKV * HD

    cosT = np.ascontiguousarray(cos.T.astype(f32))   # [64, T]
    sinT = np.ascontiguousarray(sin.T.astype(f32))
    C2 = np.empty((128, T), f16)
    S2m = np.empty((128, T), f16)
    for q in range(4):
        for i in range(32):
            j = 16 * q + (i % 16)
            C2[32 * q + i] = cosT[j]
            S2m[32 * q + i] = sinT[j] if i >= 16 else -sinT[j]
    import ml_dtypes
    ones = np.ones((128, 128), ml_dtypes.bfloat16)

    in_maps = []
    for core in range(N_CORES):
        b, g = divmod(core, 4)
        qrows = np.concatenate([(8 * g + j) * HD + perm for j in range(GQ)])
        krows = np.concatenate([(2 * g + m) * HD + perm for m in range(GKV)])
        vrows = np.arange(2 * g * HD, (2 * g + 2) * HD)
        ocols = np.arange(8 * g * HD, (8 * g + 8) * HD)
        # woT_core[r, c] = wo[c, ocols[r]]; repack so m-block slices are contiguous:
        # WO2[p, m*1024 + hk*128 + c] = woT_core[hk*128 + p, m*128 + c]
        woT_core = wo[:, ocols].T.astype(f16)
        WO2 = np.ascontiguousarray(
            woT_core.reshape(GQ, 128, D // 128, 128)
            .transpose(1, 2, 0, 3).reshape(128, (D // 128) * QD))
        in_maps.append({
            "xT": np.ascontiguousarray(x[b].T.astype(f16)),
            "wqT": np.ascontiguousarray(wq[qrows].T.astype(f16)),
            "wkvT": np.ascontiguousarray(
                np.concatenate([wk[krows], wv[vrows]], axis=0).T.astype(f16)),
            "woT": WO2,
            "C2": C2, "S2m": S2m, "ones": ones,
        })
    return in_maps


def kernel(x, wq, wk, wv, wo, cache_k, cache_v, cos, sin, mask, start_pos):
    x = np.asarray(x)
    wq, wk, wv, wo = (np.asarray(a) for a in (wq, wk, wv, wo))
    cos, sin = np.asarray(cos), np.asarray(sin)
    assert int(start_pos) == 0, "kernel hardcodes start_pos == 0"
    assert x.shape == (2, T, D)

    from concourse.bass_utils import run_bass_kernel_spmd

    nc = _get_program()
    in_maps = _host_prep(x, wq, wk, wv, wo, cos, sin)
    res = run_bass_kernel_spmd(nc, in_maps, list(range(N_CORES)))

    y = np.empty((2, T, D), np.float32)
    for b in range(2):
        acc = res.results[4 * b]["yT"].astype(np.float32)
        for g in range(1, 4):
            acc += res.results[4 * b + g]["yT"].astype(np.float32)
        y[b] = acc.T
    return y


# revision 32
# speedup vs baseline: 1.0314x; 1.0057x over previous
"""Self-contained Trainium2 Bass kernel for GQA attention (B=2, T=2048, D=4096,
32 q heads / 8 kv heads, HD=128, RoPE, no causal mask, start_pos=0).

Sharding: 8 cores = 2 (batch) x 4 (head groups). Each core computes 8 q heads /
2 kv heads for one batch and a partial o-projection; the host sums the 4
partials per batch.

All matmul operands are fp16 (FWL-enabled weight loads, 1024-col moving
operands); PSUM accumulation is fp32. exp uses bias=-2 so values fit fp16.

Per-core structure (transposed space, feature dim on partitions):
  Phase 1 (per 512-col t-chunk): qT = wqT.T @ xT (8 psum banks), RoPE evac to
    SBUF fp16; then kT (rope'd) + v in natural [t, hd] layout (x as stationary).
    wkv resident in SBUF; wq streamed per chunk; x chunk resident for both passes.
  Phase 2 (per head, per 1024-col chunk): scoresT = k_blk.T @ qT (N=1024),
    exp via one ACT op -> fp16 SBUF, den += ones.T @ exp, ctx += v.T @ exp,
    then ctx *= reciprocal_approx_fast(den).
  Phase 3: yT[m-block] = sum_h woT.T @ ctx_h (N=1024), evac fp16, DMA out.
"""

import sys
import math

for _p in ("/opt/trn_rl_repo", "/root/.axon_site"):
    if _p not in sys.path:
        sys.path.insert(0, _p)

import numpy as np

T = 2048
D = 4096
N_HEADS = 32
N_KV = 8
HD = 128
N_CORES = 8
GQ = N_HEADS // 4   # q heads per core = 8
GKV = N_KV // 4     # kv heads per core = 2
TCH = 512           # t-chunk for phase 1
TQ = 1024           # t-chunk for phases 2/3
SCALE = 1.0 / math.sqrt(HD)


def _build_program():
    import concourse.bass as bass
    import concourse.tile as tile
    from concourse import bacc, mybir

    f32 = mybir.dt.float32
    f16 = mybir.dt.float16
    bf16 = mybir.dt.bfloat16

    QD, KD, KT = GQ * HD, GKV * HD, D // 128
    N_REP = GQ // GKV

    nc = bacc.Bacc("TRN2", target_bir_lowering=False, debug=False,
                   num_devices=N_CORES)

    xT = nc.dram_tensor("xT", [D, T], f16, kind="ExternalInput")
    wqT = nc.dram_tensor("wqT", [D, QD], f16, kind="ExternalInput")
    wkvT = nc.dram_tensor("wkvT", [D, 2 * KD], f16, kind="ExternalInput")
    # wo pre-permuted on host: [128, m-block * (hk * 128)] so each m-block's
    # eight stationary [128,128] slices are one contiguous [128, 1024] DMA
    woT = nc.dram_tensor("woT", [128, (D // 128) * QD], f16,
                         kind="ExternalInput")
    C2 = nc.dram_tensor("C2", [128, T], f16, kind="ExternalInput")
    S2m = nc.dram_tensor("S2m", [128, T], f16, kind="ExternalInput")
    ones = nc.dram_tensor("ones", [128, 128], bf16, kind="ExternalInput")
    yT = nc.dram_tensor("yT", [D, T], f16, kind="ExternalOutput")

    NTCH = T // TCH           # 4
    NSB = T // 128            # 16 s-blocks for attention
    NTB = TCH // 128          # 4 v row-blocks per chunk
    SWAP = [(i + 16) % 32 for i in range(32)]  # swap 16-halves in each quadrant

    with tile.TileContext(nc) as tc:
        with tc.tile_pool(name="persist", bufs=1) as persist:
            # tiles allocated here; DMAs issued inside the chunk-0 loop so the
            # first q matmuls aren't stuck behind 6MB of weight/table loads
            ones_sb = persist.tile([128, 128], bf16, tag="ones")
            c2_sb = persist.tile([128, T], f16, tag="c2")
            s2m_sb = persist.tile([128, T], f16, tag="s2m")
            wkv_sb = [persist.tile([128, 2 * KD], f16, name=f"wkv{k}", tag=f"wkv{k}")
                      for k in range(KT)]
            # wq fully resident after chunk 0 (8.4MB fp16): chunks 1-3 run
            # pass A with no weight DMA at all
            wq_sb = [persist.tile([128, QD], f16, name=f"wq{k}", tag=f"wq{k}")
                     for k in range(KT)]
            # q / k (rope'd, [hd, t]) and v (natural [t, hd]) in SBUF fp16
            q_sb = [persist.tile([128, T], f16, name=f"q{m}", tag=f"q{m}")
                    for m in range(GQ)]
            k_sb = [persist.tile([128, T], f16, name=f"k{m}", tag=f"k{m}")
                    for m in range(GKV)]
            v_sb = [persist.tile([128, KD], bf16, name=f"v{tb}", tag=f"v{tb}")
                    for tb in range(T // 128)]

            # ---------------- Phase 1: q/k/v projections + RoPE ----------
            with tc.tile_pool(name="xt", bufs=1) as xtp, \
                 tc.tile_pool(name="rope", bufs=2) as ropep, \
                 tc.tile_pool(name="p1ps", bufs=8, space="PSUM") as p1ps:

                def rope_evac(ps, dst_ap, tcol0, tcol1):
                    # dst = ps * C2 + shuffle(ps) * S2m  (on the chunk col slice)
                    t1 = ropep.tile([128, TCH], f32, tag="t1")
                    nc.vector.tensor_mul(t1[:], ps[:], c2_sb[:, tcol0:tcol1])
                    sh = ropep.tile([128, TCH], f32, tag="sh")
                    nc.vector.stream_shuffle(sh[:], ps[:], SWAP)
                    t2 = ropep.tile([128, TCH], f32, tag="t2")
                    nc.vector.tensor_mul(t2[:], sh[:], s2m_sb[:, tcol0:tcol1])
                    nc.vector.tensor_add(dst_ap, t1[:], t2[:])

                for tch in range(NTCH):
                    tcol0, tcol1 = tch * TCH, (tch + 1) * TCH
                    xts = []
                    # pass A: q projection (8 live PSUM accumulators)
                    qps = [p1ps.tile([128, TCH], f32, name=f"qps{_}", tag="p1")
                           for _ in range(GQ)]
                    for k in range(KT):
                        xt = xtp.tile([128, TCH], f16, tag=f"xt{k}")
                        nc.sync.dma_start(xt[:], xT[k * 128:(k + 1) * 128, tcol0:tcol1])
                        xts.append(xt)
                        wq_sl = wq_sb[k]
                        if tch == 0:
                            nc.sync.dma_start(wq_sl[:], wqT[k * 128:(k + 1) * 128, :])
                            # trickle in the persistent tensors behind the
                            # chunk-0 x/wq loads (wkv needed in pass B, tables
                            # at first rope evac, ones in phase 2); delay the
                            # first few so the k=0..2 x/wq tiles go first
                            if k >= 3:
                                nc.sync.dma_start(wkv_sb[k][:],
                                                  wkvT[k * 128:(k + 1) * 128, :])
                            if k == 31:
                                for kk in range(3):
                                    nc.sync.dma_start(wkv_sb[kk][:],
                                                      wkvT[kk * 128:(kk + 1) * 128, :])
                            if k == 20:
                                nc.sync.dma_start(c2_sb[:], C2[:])
                            if k == 26:
                                nc.sync.dma_start(s2m_sb[:], S2m[:])
                            if k == 30:
                                nc.sync.dma_start(ones_sb[:], ones[:])
                        for m in range(GQ):
                            nc.tensor.matmul(qps[m][:], wq_sl[:, m * 128:(m + 1) * 128],
                                             xt[:], start=(k == 0), stop=(k == KT - 1))
                    for m in range(GQ):
                        rope_evac(qps[m], q_sb[m][:, tcol0:tcol1], tcol0, tcol1)
                    # pass B: k and v projections (x tiles reused from SBUF)
                    kps = [p1ps.tile([128, TCH], f32, name=f"kps{_}", tag="p1")
                           for _ in range(GKV)]
                    vps = [p1ps.tile([128, KD], f32, name=f"vps{_}", tag="p1")
                           for _ in range(NTB)]
                    if tch == NTCH - 1:
                        # last chunk: kps sweep first so the k rope evacs (the
                        # gate for all of phase 2) finish while the vps sweep
                        # still runs; v tiles are only needed late in phase 2
                        for k in range(KT):
                            for m in range(GKV):
                                nc.tensor.matmul(kps[m][:], wkv_sb[k][:, m * 128:(m + 1) * 128],
                                                 xts[k][:], start=(k == 0), stop=(k == KT - 1))
                        for m in range(GKV):
                            rope_evac(kps[m], k_sb[m][:, tcol0:tcol1], tcol0, tcol1)
                        for k in range(KT):
                            for tb in range(NTB):
                                nc.tensor.matmul(vps[tb][:], xts[k][:, tb * 128:(tb + 1) * 128],
                                                 wkv_sb[k][:, KD:], start=(k == 0), stop=(k == KT - 1))
                        for tb in range(NTB):
                            nc.scalar.copy(v_sb[tch * NTB + tb][:], vps[tb][:])
                    else:
                        for k in range(KT):
                            for m in range(GKV):
                                nc.tensor.matmul(kps[m][:], wkv_sb[k][:, m * 128:(m + 1) * 128],
                                                 xts[k][:], start=(k == 0), stop=(k == KT - 1))
                            for tb in range(NTB):
                                nc.tensor.matmul(vps[tb][:], xts[k][:, tb * 128:(tb + 1) * 128],
                                                 wkv_sb[k][:, KD:], start=(k == 0), stop=(k == KT - 1))
                        for m in range(GKV):
                            rope_evac(kps[m], k_sb[m][:, tcol0:tcol1], tcol0, tcol1)
                        for tb in range(NTB):
                            nc.scalar.copy(v_sb[tch * NTB + tb][:], vps[tb][:])

            # ---------------- Phase 2: attention per q head --------------
            with tc.tile_pool(name="ctxsb", bufs=1) as ctxp:
              ctx_sb = [ctxp.tile([128, T], f16, name=f"ctx{h}", tag=f"ctx{h}")
                        for h in range(GQ)]
              with tc.tile_pool(name="exp", bufs=8) as expp, \
                   tc.tile_pool(name="nrm", bufs=1) as nrmp, \
                   tc.tile_pool(name="scps", bufs=2, space="PSUM") as scps, \
                   tc.tile_pool(name="ctxps", bufs=1, space="PSUM") as ctxps, \
                   tc.tile_pool(name="denps", bufs=1, space="PSUM") as denps:
                # Software pipeline over groups of 4 s-blocks: scores+exp for
                # group g+1 are interleaved with the ctx matmuls of group g, so
                # the PE never head-of-line blocks on the scalar engine's exp.
                # The den (softmax denominator) matmuls only need one output
                # row, so each group's four are packed as M=32 column tiles at
                # col positions 0/32/64/96 -- they run concurrently on the PE
                # (quadrant i of den_ps accumulates sb = i mod 4); the four
                # partial dens are summed on DVE after 3 PSUM->SBUF DMAs.
                NG = NSB // 4
                groups = [(h, tq, g) for h in range(GQ)
                          for tq in range(T // TQ) for g in range(NG)]

                def issue_one(h, tq, g, i):
                    kv = h // N_REP
                    sb = 4 * g + i
                    sc_ps = scps.tile([128, TQ], f32, name="scp", tag="sc")
                    for j in (0, 1):
                        nc.tensor.matmul(sc_ps[:, j * 512:(j + 1) * 512],
                                         k_sb[kv][:, sb * 128:(sb + 1) * 128],
                                         q_sb[h][:, tq * TQ + j * 512:tq * TQ + (j + 1) * 512],
                                         start=True, stop=True)
                    ex = expp.tile([128, TQ], bf16, name="exp", tag="ex")
                    nc.scalar.activation(ex[:], sc_ps[:],
                                         mybir.ActivationFunctionType.Exp,
                                         scale=SCALE)
                    return ex

                acc = {}   # (h, tq) -> (ctx_ps, den_ps)
                exs_pend = [issue_one(*groups[0], i) for i in range(4)]
                for gi, (h, tq, g) in enumerate(groups):
                    kv = h // N_REP
                    exs = exs_pend
                    exs_pend = []
                    if g == 0:
                        acc[(h, tq)] = (ctxps.tile([128, TQ], f32, name="ctxp", tag="ctx"),
                                        denps.tile([128, TQ], f32, name="denp", tag="den"))
                    ctx_ps, den_ps = acc[(h, tq)]
                    for i in range(4):
                        sb = 4 * g + i
                        if gi + 1 < len(groups):
                            # next group's scores+exp trickle in one sb at a
                            # time so the scalar engine never starves
                            exs_pend.append(issue_one(*groups[gi + 1], i))
                        for j in (0, 1):
                            nc.tensor.matmul(ctx_ps[:, j * 512:(j + 1) * 512],
                                             v_sb[sb][:, kv * 128:(kv + 1) * 128],
                                             exs[i][:, j * 512:(j + 1) * 512],
                                             start=(sb == 0), stop=(sb == NSB - 1))
                    # packed den: M=64 col tiles at positions 0/64, split by
                    # j-half -- row-half j accumulates columns j*512:(j+1)*512
                    # over ALL sb, so each half is a complete denominator for
                    # its columns (no cross-half fold needed). The two MMs per
                    # i are adjacent -> run concurrently on the PE.
                    for i in range(4):
                        for j in (0, 1):
                            nc.tensor.matmul(den_ps[64 * j:64 * (j + 1), j * 512:(j + 1) * 512],
                                             ones_sb[:, 0:64],
                                             exs[i][:, j * 512:(j + 1) * 512],
                                             start=(g == 0 and i == 0),
                                             stop=(g == NG - 1 and i == 3))
                    if g == NG - 1:
                        # reciprocal per j-half straight off PSUM, then
                        # normalize ctx per 64-partition half (rb lives on
                        # partitions 0:64, no broadcast needed)
                        dtmp = nrmp.tile([64, 512], f32, name="dtmp", tag="dtmp")
                        nc.vector.tensor_copy(dtmp[:], den_ps[64:128, 512:1024])
                        rb = nrmp.tile([64, TQ], f32, name="rb", tag="rb")
                        nc.vector.reciprocal_approx_fast(
                            rb[:, 0:512], den_ps[0:64, 0:512])
                        nc.vector.reciprocal_approx_fast(
                            rb[:, 512:1024], dtmp[:])
                        for pb in (0, 64):
                            nc.vector.tensor_mul(
                                ctx_sb[h][pb:pb + 64, tq * TQ:(tq + 1) * TQ],
                                ctx_ps[pb:pb + 64, :], rb[:])
                        del acc[(h, tq)]

              # ------------- Phase 3: o-projection (ctx in SBUF) -------
              with tc.tile_pool(name="wo", bufs=3) as wop, \
                   tc.tile_pool(name="out", bufs=4) as outp, \
                   tc.tile_pool(name="yps", bufs=3, space="PSUM") as yps:
                  for m in range(D // 128):
                      # all 8 head-blocks of wo for this m-block in one DMA
                      wo_m = wop.tile([128, QD], f16, tag="wo")
                      nc.sync.dma_start(wo_m[:], woT[:, m * QD:(m + 1) * QD])
                      for tq in range(T // TQ):
                          y_ps = yps.tile([128, TQ], f32, tag="y")
                          for hk in range(GQ):
                              for j in (0, 1):
                                  nc.tensor.matmul(
                                      y_ps[:, j * 512:(j + 1) * 512],
                                      wo_m[:, hk * 128:(hk + 1) * 128],
                                      ctx_sb[hk][:, tq * TQ + j * 512:tq * TQ + (j + 1) * 512],
                                      start=(hk == 0), stop=(hk == GQ - 1))
                          ot = outp.tile([128, TQ], f16, tag="ot")
                          if tq % 2 == 0:
                              nc.vector.tensor_copy(ot[:], y_ps[:])
                          else:
                              nc.scalar.copy(ot[:], y_ps[:])
                          nc.sync.dma_start(yT[m * 128:(m + 1) * 128,
                                               tq * TQ:(tq + 1) * TQ], ot[:])

    nc.compile()
    return nc


_PROGRAM = None


def _get_program():
    global _PROGRAM
    if _PROGRAM is None:
        _PROGRAM = _build_program()
    return _PROGRAM


def _rope_perm():
    """Within-head row permutation: row 32*q + i  <-  component 2*(16q+i%16)+ (i>=16)."""
    perm = np.empty(HD, dtype=np.int64)
    for q in range(4):
        for i in range(32):
            j = 16 * q + (i % 16)
            perm[32 * q + i] = 2 * j + (1 if i >= 16 else 0)
    return perm


def _host_prep(x, wq, wk, wv, wo, cos, sin):
    """Build the per-core input maps."""
    perm = _rope_perm()
    f32 = np.float32
    f16 = np.float16
    QD, KD = GQ * HD, GKV * HD

    cosT = np.ascontiguousarray(cos.T.astype(f32))   # [64, T]
    sinT = np.ascontiguousarray(sin.T.astype(f32))
    C2 = np.empty((128, T), f16)
    S2m = np.empty((128, T), f16)
    for q in range(4):
        for i in range(32):
            j = 16 * q + (i % 16)
            C2[32 * q + i] = cosT[j]
            S2m[32 * q + i] = sinT[j] if i >= 16 else -sinT[j]
    import ml_dtypes
    ones = np.ones((128, 128), ml_dtypes.bfloat16)

    in_maps = []
    for core in range(N_CORES):
        b, g = divmod(core, 4)
        qrows = np.concatenate([(8 * g + j) * HD + perm for j in range(GQ)])
        krows = np.concatenate([(2 * g + m) * HD + perm for m in range(GKV)])
        vrows = np.arange(2 * g * HD, (2 * g + 2) * HD)
        ocols = np.arange(8 * g * HD, (8 * g + 8) * HD)
        # woT_core[r, c] = wo[c, ocols[r]]; repack so m-block slices are contiguous:
        # WO2[p, m*1024 + hk*128 + c] = woT_core[hk*128 + p, m*128 + c]
        woT_core = wo[:, ocols].T.astype(f16)
        WO2 = np.ascontiguousarray(
            woT_core.reshape(GQ, 128, D // 128, 128)
            .transpose(1, 2, 0, 3).reshape(128, (D // 128) * QD))
        in_maps.append({
            "xT": np.ascontiguousarray(x[b].T.astype(f16)),
            "wqT": np.ascontiguousarray(wq[qrows].T.astype(f16)),
            "wkvT": np.ascontiguousarray(
                np.concatenate([wk[krows], wv[vrows]], axis=0).T.astype(f16)),
            "woT": WO2,
            "C2": C2, "S2m": S2m, "ones": ones,
        })
    return in_maps


def kernel(x, wq, wk, wv, wo, cache_k, cache_v, cos, sin, mask, start_pos):
    x = np.asarray(x)
    wq, wk, wv, wo = (np.asarray(a) for a in (wq, wk, wv, wo))
    cos, sin = np.asarray(cos), np.asarray(sin)
    assert int(start_pos) == 0, "kernel hardcodes start_pos == 0"
    assert x.shape == (2, T, D)

    from concourse.bass_utils import run_bass_kernel_spmd

    nc = _get_program()
    in_maps = _host_prep(x, wq, wk, wv, wo, cos, sin)
    res = run_bass_kernel_spmd(nc, in_maps, list(range(N_CORES)))

    y = np.empty((2, T, D), np.float32)
    for b in range(2):
        acc = res.results[4 * b]["yT"].astype(np.float32)
        for g in range(1, 4):
            acc += res.results[4 * b + g]["yT"].astype(np.float32)
        y[b] = acc.T
    return y
